# revision 31
# baseline (speedup 1.0000x reference)
"""Trainium2 Bass kernel for 16-head MultiHeadAttention (B=2, S=2048, D=1024).

Strategy: tensor-parallel over heads. 8 cores x 2 heads each. Each core:
  - receives host-pre-transposed qT/kT/vT [B, D, S] (full), plus its head-slice
    of w_q/w_k/w_v (columns) and w_o (rows),
  - computes Q^T/K^T/V^T projections for its 2 heads (fp32r matmuls,
    contraction over D with weight Dtiles as the stationary operand),
  - V^T is PE-transposed back to V-natural [tok, d] with an appended ones
    column (so the attnV matmul also yields softmax row-sums for free),
  - attention per (batch, head):
      natural layout:  logits[q,k] -> exp (ACT, scale=1/8 folded, accum_out
                       row-sums) -> normalize (DVE per-partition scalar) ->
                       DMA out attn_weights,
      transposed:      logits^T[k,q] -> exp -> attnV accumulation
                       (lhsT=[V|ones]) -> per-q normalization via
                       reciprocal + gpsimd partition_broadcast,
  - output projection partial = attnV_norm^T @ w_o_slice.

attn_weights are written to HBM (and shipped to the host) as bf16 and upcast
host-side (~1.8e-3 rel err vs 3e-4 for fp32); set KERNEL_ATTN_FP32=1 to keep
them fp32 end-to-end. Emission order interleaves batch-1 projections into
batch-0's attention so input DMA hides under ACT-bound softmax work
(cost-model estimate ~416us/core). The runner ships replicated inputs once
(sharded + on-device all-gather), creates donated zero output buffers on
device, and reduces the 8 partial outputs on device; host adds b_o and
concatenates heads.
"""

import os
import sys

sys.path.insert(0, "/opt/trn_rl_repo")

import numpy as np

import concourse.bass as bass
import concourse.mybir as mybir
import concourse.tile as tile
from concourse import bacc
from concourse.masks import make_identity

FP32 = mybir.dt.float32
F32R = mybir.dt.float32r
AF = mybir.ActivationFunctionType

# Full-problem constants (hardcoded per contest contract).
B, S, D = 2, 2048, 1024
N_HEADS = 16
DEPTH = D // N_HEADS  # 64
N_CORES = 8
HPC = N_HEADS // N_CORES  # heads per core = 2
DSL = HPC * DEPTH  # per-core projection slice width = 128


def _r(ap):
    """fp32 tile viewed as float32r for the fast PE path."""
    return ap.bitcast(F32R)


def _chunks(total, size):
    out = []
    o = 0
    while o < total:
        c = min(size, total - o)
        out.append((o, c))
        o += c
    return out


def build_program(b=B, s=S, d=D, use_f32r=True, nat_side=True, attn_bf16=True):
    """Build the per-core Bass program. Returns nc."""
    nc = bacc.Bacc("TRN2", target_bir_lowering=False, debug=False)

    MDT = F32R if use_f32r else FP32  # dtype for matmul-feeding tensors
    qT = nc.dram_tensor("qT", [b, d, s], MDT, kind="ExternalInput").ap()
    kT = nc.dram_tensor("kT", [b, d, s], MDT, kind="ExternalInput").ap()
    vT = nc.dram_tensor("vT", [b, d, s], MDT, kind="ExternalInput").ap()
    wq = nc.dram_tensor("wq", [d, DSL], MDT, kind="ExternalInput").ap()
    wk = nc.dram_tensor("wk", [d, DSL], MDT, kind="ExternalInput").ap()
    wv = nc.dram_tensor("wv", [d, DSL], MDT, kind="ExternalInput").ap()
    bq = nc.dram_tensor("bq", [DSL, 1], FP32, kind="ExternalInput").ap()
    bk = nc.dram_tensor("bk", [DSL, 1], FP32, kind="ExternalInput").ap()
    bv = nc.dram_tensor("bv", [DSL, 1], FP32, kind="ExternalInput").ap()
    wo = nc.dram_tensor("wo", [DSL, d], MDT, kind="ExternalInput").ap()
    ones = nc.dram_tensor("ones", [128, s // 128, 1], MDT, kind="ExternalInput").ap()

    ADT = mybir.dt.bfloat16 if attn_bf16 else FP32
    attn_w = nc.dram_tensor("attn_w", [b, HPC, s, s], ADT, kind="ExternalOutput").ap()
    out_p = nc.dram_tensor("out_p", [b, s, d], FP32, kind="ExternalOutput").ap()

    n_dt = d // 128  # D tiles (contraction)
    n_kt = s // 128  # key tiles
    scale = 1.0 / np.sqrt(np.float32(DEPTH))

    mm_cast = lambda ap: ap

    from contextlib import ExitStack

    with tile.TileContext(nc) as tc, ExitStack() as ctx:
        # ---- pools ----
        persist = ctx.enter_context(tc.tile_pool(name="persist", bufs=1))
        xin = ctx.enter_context(tc.tile_pool(name="xin", bufs=6))
        vtp = ctx.enter_context(tc.tile_pool(name="vtp", bufs=2))
        natp = ctx.enter_context(tc.tile_pool(name="natp", bufs=2))
        sbtp = ctx.enter_context(tc.tile_pool(name="sbtp", bufs=3))
        smallp = ctx.enter_context(tc.tile_pool(name="smallp", bufs=2))
        pp_acc = ctx.enter_context(tc.tile_pool(name="pp_acc", bufs=2, space="PSUM"))
        pp_acc2 = ctx.enter_context(tc.tile_pool(name="pp_acc2", bufs=2, space="PSUM"))
        pp_av = ctx.enter_context(tc.tile_pool(name="pp_av", bufs=1, space="PSUM"))
        pp_out = ctx.enter_context(tc.tile_pool(name="pp_out", bufs=1, space="PSUM"))

        # ---- constants / weights ----
        ident = persist.tile([128, 128], FP32, tag="ident", name="ident")
        make_identity(nc, ident[:])

        w_sb = {}
        for name, wap in (("q", wq), ("k", wk), ("v", wv)):
            t = persist.tile([128, n_dt, DSL], MDT, tag=f"w{name}", name=f"w{name}_sb")
            nc.sync.dma_start(t[:], wap.rearrange("(t p) n -> p t n", p=128))
            w_sb[name] = t
        b_sb = {}
        for name, bap in (("q", bq), ("k", bk), ("v", bv)):
            t = persist.tile([DSL, 1], FP32, tag=f"b{name}", name=f"b{name}_sb")
            nc.sync.dma_start(t[:], bap)
            b_sb[name] = t
        wo_sb = []
        for h in range(HPC):
            t = persist.tile([DEPTH, d], MDT, tag=f"wo{h}", name=f"wo{h}_sb")
            nc.sync.dma_start(t[:], wo[h * DEPTH : (h + 1) * DEPTH, :])
            wo_sb.append(t)

        # persistent per-batch projected tensors
        QTs = [persist.tile([DSL, s], MDT, tag=f"qt{bi}", name=f"qt{bi}_sb") for bi in range(b)]
        KTs = [persist.tile([DSL, s], MDT, tag=f"kt{bi}", name=f"kt{bi}_sb") for bi in range(b)]
        # V natural [ktok, d] per (b, h), 16 tiles of [128, 65]; col 64 = ones
        Vn = [
            [persist.tile([128, n_kt, DEPTH + 1], MDT, tag=f"vn{bi}{h}", name=f"vn{bi}{h}_sb")
             for h in range(HPC)]
            for bi in range(b)
        ]
        # normalized attnV^T [d, s] per (b, h)
        avn = [
            [persist.tile([DEPTH, s], MDT, tag=f"avn{bi}{h}", name=f"avn{bi}{h}_sb") for h in range(HPC)]
            for bi in range(b)
        ]

        for bi in range(b):
            for h in range(HPC):
                nc.sync.dma_start(Vn[bi][h][:, :, DEPTH : DEPTH + 1], ones)

        # ---- helpers ----
        vt_cur = {}

        def proj_tensor(bi, name, half=None):
            """Project one of q/k/v for batch bi. half=0/1 restricts to one
            half of the sequence (strips are loaded per-half to keep DMA
            bursts small enough to interleave under attention)."""
            xap = {"q": qT, "k": kT, "v": vT}[name]
            if name == "v" and half in (None, 0):
                vt_cur[bi] = vtp.tile([DSL, s], FP32, tag="vt", name="vt_sb")
            vt_sb = vt_cur.get(bi)
            halves = [(0, s // 2), (s // 2, s // 2)] if half is None else (
                [(0, s // 2)] if half == 0 else [(s // 2, s - s // 2)]
            )
            for ho, hl in halves:
                strips = []
                for t in range(n_dt):
                    st = xin.tile([128, s // 2], MDT, tag="strip", name="strip")
                    nc.sync.dma_start(
                        st[:, :hl], xap[bi, t * 128 : (t + 1) * 128, ho : ho + hl]
                    )
                    strips.append(st)
                for co, cl in _chunks(hl, 512):
                    ps = pp_acc.tile([128, 512], FP32, tag="acc", name="acc_ps")
                    for t in range(n_dt):
                        nc.tensor.matmul(
                            ps[:, :cl],
                            mm_cast(w_sb[name][:, t, :]),
                            mm_cast(strips[t][:, co : co + cl]),
                            start=(t == 0),
                            stop=(t == n_dt - 1),
                        )
                    dst = {"q": QTs[bi], "k": KTs[bi], "v": vt_sb}[name]
                    nc.vector.tensor_scalar_add(
                        dst[:, ho + co : ho + co + cl], ps[:, :cl],
                        b_sb[name][:, 0:1],
                    )
                # V^T -> V natural (per head), via PE transpose
                if name == "v":
                    for kt in range(ho // 128, (ho + hl) // 128):
                        pvt = pp_acc.tile([128, 128], FP32, tag="acc", name="accT_ps")
                        nc.tensor.transpose(
                            pvt[:], vt_sb[:, kt * 128 : (kt + 1) * 128], ident[:]
                        )
                        for h in range(HPC):
                            nc.vector.tensor_copy(
                                Vn[bi][h][:, kt, 0:DEPTH],
                                pvt[:, h * DEPTH : (h + 1) * DEPTH],
                            )

        def attn_head(bi, h, chunk_hooks=None):
            chunk_hooks = list(chunk_hooks or [])
            if True:
                qh = QTs[bi][h * DEPTH : (h + 1) * DEPTH, :]
                kh = KTs[bi][h * DEPTH : (h + 1) * DEPTH, :]

                # transposed side: logits^T -> exp -> attnV (+rowsum)
                def t_side(qo, ql):
                    av_ps = pp_av.tile([DEPTH + 1, 512], FP32, tag="av", name="av_ps")
                    for kt2 in range(n_kt // 2):
                        t_ps = pp_acc2.tile([128, 2, 512], FP32, tag="acc2", name="acc2_ps")
                        for j in range(2):
                            kt = 2 * kt2 + j
                            nc.tensor.matmul(
                                t_ps[:, j, :ql],
                                mm_cast(kh[:, kt * 128 : (kt + 1) * 128]),
                                mm_cast(qh[:, qo : qo + ql]),
                                start=True,
                                stop=True,
                            )
                        sbt = sbtp.tile([128, 2, 512], MDT, tag="sbt", name="sbt", bufs=2)
                        nc.scalar.activation(
                            sbt[:, :, :ql], t_ps[:, :, :ql], AF.Exp,
                            scale=float(scale),
                        )
                        for j in range(2):
                            kt = 2 * kt2 + j
                            nc.tensor.matmul(
                                av_ps[:, :ql],
                                mm_cast(Vn[bi][h][:, kt, :]),
                                mm_cast(sbt[:, j, :ql]),
                                start=(kt == 0),
                                stop=(kt == n_kt - 1),
                            )
                    rT = smallp.tile([DEPTH + 1, 512], FP32, tag="rT", name="rT")
                    nc.vector.reciprocal(
                        rT[DEPTH : DEPTH + 1, :ql], av_ps[DEPTH : DEPTH + 1, :ql]
                    )
                    # partition_broadcast ucode only reads partition 0 -> hop
                    # the recip row from partition 64 to 0 via sbuf-sbuf DMA
                    r0 = smallp.tile([1, 512], FP32, tag="r0", name="r0")
                    nc.gpsimd.dma_start(r0[0:1, :ql], rT[DEPTH : DEPTH + 1, :ql])
                    rb = smallp.tile([DEPTH, 512], FP32, tag="rb", name="rb")
                    nc.gpsimd.partition_broadcast(rb[:, :ql], r0[0:1, :ql])
                    nc.vector.tensor_mul(
                        avn[bi][h][:, qo : qo + ql],
                        av_ps[0:DEPTH, :ql],
                        rb[:, :ql],
                    )

                # natural side: logits -> exp(+rowsum) -> normalize -> DMA
                n_ch = _chunks(s, 512)

                def nat_qtile(qt):
                        nexp = natp.tile([128, s], FP32, tag="nexp", name="nexp")
                        nrm = natp.tile([128, s], ADT, tag="nrm", name="nrm")
                        n_ch2 = _chunks(s, 1024)
                        rs4 = smallp.tile([128, len(n_ch2)], FP32, tag="rs4", name="rs4")
                        for ci, (co, cl) in enumerate(n_ch2):
                            n_ps = pp_acc2.tile([128, 2, 512], FP32, tag="acc2", name="acc2_ps")
                            for j in range(cl // 512):
                                nc.tensor.matmul(
                                    n_ps[:, j, :],
                                    mm_cast(qh[:, qt * 128 : (qt + 1) * 128]),
                                    mm_cast(kh[:, co + j * 512 : co + (j + 1) * 512]),
                                    start=True,
                                    stop=True,
                                )
                            nc.scalar.activation(
                                nexp[:, co : co + cl].rearrange("p (a c) -> p a c", c=512),
                                n_ps[:, : cl // 512, :], AF.Exp,
                                scale=float(scale),
                                accum_out=rs4[:, ci : ci + 1],
                            )
                        rs1 = smallp.tile([128, 1], FP32, tag="rs1", name="rs1")
                        if len(n_ch2) > 1:
                            nc.vector.reduce_sum(
                                rs1[:], rs4[:], axis=mybir.AxisListType.X
                            )
                        else:
                            rs1 = rs4
                        rr = smallp.tile([128, 1], FP32, tag="rr", name="rr")
                        nc.vector.reciprocal(rr[:], rs1[:, 0:1])
                        nc.vector.tensor_scalar_mul(nrm[:], nexp[:], rr[:, 0:1])
                        nc.gpsimd.dma_start(
                            attn_w[bi, h, qt * 128 : (qt + 1) * 128, :], nrm[:]
                        )

                qts_per_chunk = (s // 128) // len(_chunks(s, 512))
                for ci, (qo, ql) in enumerate(_chunks(s, 512)):
                    t_side(qo, ql)
                    if nat_side:
                        for qt in range(ci * qts_per_chunk, (ci + 1) * qts_per_chunk):
                            nat_qtile(qt)
                    hook = chunk_hooks.pop(0) if chunk_hooks else None
                    if hook is not None:
                        hook()

        def outproj_batch(bi):
            for tt in range(s // 128):
                for co, cl in _chunks(d, 512):
                    po = pp_out.tile([128, 512], FP32, tag="po", name="po_ps")
                    for h in range(HPC):
                        nc.tensor.matmul(
                            po[:, :cl],
                            mm_cast(avn[bi][h][:, tt * 128 : (tt + 1) * 128]),
                            mm_cast(wo_sb[h][:, co : co + cl]),
                            start=(h == 0),
                            stop=(h == HPC - 1),
                        )
                    po_sb = sbtp.tile([128, 512], FP32, tag="po_sb", name="po_sb")
                    nc.vector.tensor_copy(po_sb[:, :cl], po[:, :cl])
                    nc.sync.dma_start(
                        out_p[bi, tt * 128 : (tt + 1) * 128, co : co + cl],
                        po_sb[:, :cl],
                    )

        for nm in ("k", "q", "v"):
            proj_tensor(0, nm)
        if b > 1:
            attn_head(0, 0, [
                lambda: proj_tensor(1, "k", 0),
                lambda: proj_tensor(1, "k", 1),
                lambda: proj_tensor(1, "q", 0),
                lambda: proj_tensor(1, "q", 1),
            ])
            attn_head(0, 1, [
                lambda: proj_tensor(1, "v", 0),
                lambda: proj_tensor(1, "v", 1),
            ])
            outproj_batch(0)
            attn_head(1, 0)
            attn_head(1, 1)
            outproj_batch(1)
        else:
            attn_head(0, 0)
            attn_head(0, 1)
            outproj_batch(0)

    nc.compile()
    return nc


_NC_CACHE = {}
LAST_EXEC_NS = None


def _get_program():
    attn_bf16 = not int(os.environ.get("KERNEL_ATTN_FP32", "0"))
    key = (B, S, D, attn_bf16)
    if key not in _NC_CACHE:
        _NC_CACHE[key] = build_program(attn_bf16=attn_bf16)
    return _NC_CACHE[key]


def kernel(**inputs):
    q = np.asarray(inputs["q"], np.float32)
    k = np.asarray(inputs["k"], np.float32)
    v = np.asarray(inputs["v"], np.float32)
    w_q = np.asarray(inputs["w_q"], np.float32)
    w_k = np.asarray(inputs["w_k"], np.float32)
    w_v = np.asarray(inputs["w_v"], np.float32)
    b_q = np.asarray(inputs["b_q"], np.float32)
    b_k = np.asarray(inputs["b_k"], np.float32)
    b_v = np.asarray(inputs["b_v"], np.float32)
    w_o = np.asarray(inputs["w_o"], np.float32)
    b_o = np.asarray(inputs["b_o"], np.float32)

    qT = np.ascontiguousarray(q.transpose(0, 2, 1))
    kTn = np.ascontiguousarray(k.transpose(0, 2, 1))
    vTn = np.ascontiguousarray(v.transpose(0, 2, 1))

    in_maps = []
    for c in range(N_CORES):
        sl = slice(c * DSL, (c + 1) * DSL)
        in_maps.append(
            {
                "qT": qT,
                "kT": kTn,
                "vT": vTn,
                "wq": np.ascontiguousarray(w_q[:, sl]),
                "wk": np.ascontiguousarray(w_k[:, sl]),
                "wv": np.ascontiguousarray(w_v[:, sl]),
                "bq": np.ascontiguousarray(b_q[sl, None]),
                "bk": np.ascontiguousarray(b_k[sl, None]),
                "bv": np.ascontiguousarray(b_v[sl, None]),
                "wo": np.ascontiguousarray(w_o[sl, :]),
                "ones": np.ones((128, S // 128, 1), np.float32),
            }
        )

    nc = _get_program()
    if int(os.environ.get("KERNEL_SLOW_RUNNER", "0")):
        from concourse.bass_utils import run_bass_kernel_spmd

        r = run_bass_kernel_spmd(nc, in_maps, core_ids=list(range(N_CORES)))
        res = r.results
        attn = np.empty((B, N_HEADS, S, S), np.float32)
        acc = np.zeros((B, S, D), np.float64)
        for c in range(N_CORES):
            attn[:, c * HPC : (c + 1) * HPC] = res[c]["attn_w"]
            acc += res[c]["out_p"]
        output = (acc + b_o.astype(np.float64)).astype(np.float32)
        return output, attn

    try:
        attn_g, outp = _run_fast(nc, in_maps)
    except Exception:
        # fall back to the stock SPMD runner on any fast-path failure
        from concourse.bass_utils import run_bass_kernel_spmd

        res = run_bass_kernel_spmd(nc, in_maps, core_ids=list(range(N_CORES))).results
        attn = np.empty((B, N_HEADS, S, S), np.float32)
        acc = np.zeros((B, S, D), np.float64)
        for c in range(N_CORES):
            attn[:, c * HPC : (c + 1) * HPC] = np.asarray(
                res[c]["attn_w"]
            ).astype(np.float32)
            acc += res[c]["out_p"]
        output = (acc + b_o.astype(np.float64)).astype(np.float32)
        return output, attn
    attn = np.ascontiguousarray(
        attn_g.reshape(N_CORES, B, HPC, S, S).transpose(1, 0, 2, 3, 4).reshape(
            B, N_HEADS, S, S
        ).astype(np.float32, copy=False)
    )
    output = (outp.astype(np.float64) + b_o.astype(np.float64)).astype(np.float32)
    return output, attn


_FAST_CACHE = {}


def _run_fast(nc, in_maps):
    """Axon PJRT runner tuned for transfer volume: replicated inputs are
    shipped once and broadcast on-device, donated output buffers are
    created on-device, and the per-core output partials are reduced
    on-device so only one copy comes back."""
    import jax
    import jax.numpy as jnp
    from jax.sharding import Mesh, NamedSharding, PartitionSpec as P
    from jax.experimental.shard_map import shard_map
    from concourse import bass2jax
    from concourse.bass2jax import install_neuronx_cc_hook
    import concourse.mybir as mybir

    install_neuronx_cc_hook()

    REPL = ("qT", "kT", "vT", "ones")

    in_names = []
    out_names = []
    out_avals = []
    part_name = nc.partition_id_tensor.name if nc.partition_id_tensor else None
    for alloc in nc.m.functions[0].allocations:
        if not isinstance(alloc, mybir.MemoryLocationSet):
            continue
        name = alloc.memorylocations[0].name
        if alloc.kind == "ExternalInput":
            if name != part_name:
                in_names.append(name)
        elif alloc.kind == "ExternalOutput":
            out_names.append(name)
            out_avals.append(
                jax.core.ShapedArray(tuple(alloc.tensor_shape), mybir.dt.np(alloc.dtype))
            )
    n_params = len(in_names)
    all_names = in_names + out_names
    if part_name is not None:
        all_names = all_names + [part_name]

    devices = jax.devices()[:N_CORES]
    mesh = Mesh(np.asarray(devices), ("core",))
    shard = NamedSharding(mesh, P("core"))
    repl = NamedSharding(mesh, P())

    def _body(*args):
        operands = list(args)
        if part_name is not None:
            operands.append(bass2jax.partition_id_tensor())
        outs = bass2jax._bass_exec_p.bind(
            *operands,
            out_avals=tuple(out_avals),
            in_names=tuple(all_names),
            out_names=tuple(out_names),
            lowering_input_output_aliases=(),
            sim_require_finite=True,
            sim_require_nnan=True,
            nc=nc,
        )
        return tuple(outs)

    in_specs = tuple(P() if n in REPL else P("core") for n in in_names) + tuple(
        P("core") for _ in out_names
    )
    out_specs = tuple(P("core") for _ in out_names)
    donate = tuple(range(n_params, n_params + len(out_names)))
    sharded = _FAST_CACHE.get("jit")
    if sharded is None:
        sharded = jax.jit(
            shard_map(
                _body, mesh=mesh, in_specs=in_specs, out_specs=out_specs,
                check_rep=False,
            ),
            donate_argnums=donate,
            keep_unused=True,
        )
        _FAST_CACHE["jit"] = sharded

    # stage inputs; replicated big tensors go over the wire once (sharded on a
    # leading axis of 8) and are all-gathered on device into replicated form
    gat = _FAST_CACHE.get("gather_jit")
    if gat is None:
        def _gather(x):
            # x: [8, b, d/8, s] sharded on axis 0 -> [b, d, s] replicated
            xb, bb, dd, ss = x.shape
            return jnp.transpose(x, (1, 0, 2, 3)).reshape(bb, xb * dd, ss)
        gat = jax.jit(_gather, out_shardings=repl)
        _FAST_CACHE["gather_jit"] = gat

    args = []
    for n in in_names:
        if n in REPL:
            v = in_maps[0][n]
            if v.ndim == 3 and v.shape[1] % N_CORES == 0 and v.nbytes >= 2**22:
                piece = v.shape[1] // N_CORES
                vp = np.ascontiguousarray(
                    v.reshape(v.shape[0], N_CORES, piece, v.shape[2]).transpose(
                        1, 0, 2, 3
                    )
                )
                args.append(gat(jax.device_put(vp, shard)))
            else:
                args.append(jax.device_put(v, repl))
        else:
            cat = np.concatenate([in_maps[c][n] for c in range(N_CORES)], axis=0)
            args.append(jax.device_put(cat, shard))
    # donated zero output buffers, created on device
    zmaker = _FAST_CACHE.get("zeros_jit")
    if zmaker is None:
        def _mk():
            return tuple(
                jnp.zeros((N_CORES * av.shape[0], *av.shape[1:]), av.dtype)
                for av in out_avals
            )
        zmaker = jax.jit(_mk, out_shardings=tuple(shard for _ in out_avals))
        _FAST_CACHE["zeros_jit"] = zmaker
    zeros = zmaker()

    outs = sharded(*args, *zeros)
    out_by_name = dict(zip(out_names, outs))

    # on-device reduction of the per-core output partials
    red = _FAST_CACHE.get("red_jit")
    if red is None:
        def _red(x):
            return jnp.sum(x.reshape(N_CORES, B, S, D), axis=0)
        red = jax.jit(_red, out_shardings=NamedSharding(mesh, P()))
        _FAST_CACHE["red_jit"] = red
    outp_dev = red(out_by_name["out_p"])

    attn_g = np.asarray(out_by_name["attn_w"])
    outp = np.asarray(outp_dev.addressable_shards[0].data)
    return attn_g, outp


# revision 32
# speedup vs baseline: 1.0027x; 1.0027x over previous
"""Trainium2 Bass kernel for 16-head MultiHeadAttention (B=2, S=2048, D=1024).

Strategy: tensor-parallel over heads. 8 cores x 2 heads each. Each core:
  - receives host-pre-transposed qT/kT/vT [B, D, S] (full), plus its head-slice
    of w_q/w_k/w_v (columns) and w_o (rows),
  - computes Q^T/K^T/V^T projections for its 2 heads (fp32r matmuls,
    contraction over D with weight Dtiles as the stationary operand),
  - V^T is PE-transposed back to V-natural [tok, d] with an appended ones
    column (so the attnV matmul also yields softmax row-sums for free),
  - attention per (batch, head):
      natural layout:  logits[q,k] -> exp (ACT, scale=1/8 folded, accum_out
                       row-sums) -> normalize (DVE per-partition scalar) ->
                       DMA out attn_weights,
      transposed:      logits^T[k,q] -> exp -> attnV accumulation
                       (lhsT=[V|ones]) -> per-q normalization via
                       reciprocal + gpsimd partition_broadcast,
  - output projection partial = attnV_norm^T @ w_o_slice.

attn_weights are written to HBM (and shipped to the host) as bf16 and upcast
host-side (~1.8e-3 rel err vs 3e-4 for fp32); set KERNEL_ATTN_FP32=1 to keep
them fp32 end-to-end. Emission order interleaves batch-1 projections into
batch-0's attention so input DMA hides under ACT-bound softmax work
(cost-model estimate ~416us/core). The runner ships replicated inputs once
(sharded + on-device all-gather), creates donated zero output buffers on
device, and reduces the 8 partial outputs on device; host adds b_o and
concatenates heads.
"""

import os
import sys

sys.path.insert(0, "/opt/trn_rl_repo")

import numpy as np

import concourse.bass as bass
import concourse.mybir as mybir
import concourse.tile as tile
from concourse import bacc
from concourse.masks import make_identity

FP32 = mybir.dt.float32
F32R = mybir.dt.float32r
AF = mybir.ActivationFunctionType

# Full-problem constants (hardcoded per contest contract).
B, S, D = 2, 2048, 1024
N_HEADS = 16
DEPTH = D // N_HEADS  # 64
N_CORES = 8
HPC = N_HEADS // N_CORES  # heads per core = 2
DSL = HPC * DEPTH  # per-core projection slice width = 128


def _r(ap):
    """fp32 tile viewed as float32r for the fast PE path."""
    return ap.bitcast(F32R)


def _chunks(total, size):
    out = []
    o = 0
    while o < total:
        c = min(size, total - o)
        out.append((o, c))
        o += c
    return out


def build_program(b=B, s=S, d=D, use_f32r=True, nat_side=True, attn_bf16=True):
    """Build the per-core Bass program. Returns nc."""
    nc = bacc.Bacc("TRN2", target_bir_lowering=False, debug=False)

    MDT = F32R if use_f32r else FP32  # dtype for matmul-feeding tensors
    qT = nc.dram_tensor("qT", [b, d, s], MDT, kind="ExternalInput").ap()
    kT = nc.dram_tensor("kT", [b, d, s], MDT, kind="ExternalInput").ap()
    vT = nc.dram_tensor("vT", [b, d, s], MDT, kind="ExternalInput").ap()
    wq = nc.dram_tensor("wq", [d, DSL], MDT, kind="ExternalInput").ap()
    wk = nc.dram_tensor("wk", [d, DSL], MDT, kind="ExternalInput").ap()
    wv = nc.dram_tensor("wv", [d, DSL], MDT, kind="ExternalInput").ap()
    bq = nc.dram_tensor("bq", [DSL, 1], FP32, kind="ExternalInput").ap()
    bk = nc.dram_tensor("bk", [DSL, 1], FP32, kind="ExternalInput").ap()
    bv = nc.dram_tensor("bv", [DSL, 1], FP32, kind="ExternalInput").ap()
    wo = nc.dram_tensor("wo", [DSL, d], MDT, kind="ExternalInput").ap()
    ones = nc.dram_tensor("ones", [128, s // 128, 1], MDT, kind="ExternalInput").ap()

    ADT = mybir.dt.bfloat16 if attn_bf16 else FP32
    attn_w = nc.dram_tensor("attn_w", [b, HPC, s, s], ADT, kind="ExternalOutput").ap()
    out_p = nc.dram_tensor("out_p", [b, s, d], FP32, kind="ExternalOutput").ap()

    n_dt = d // 128  # D tiles (contraction)
    n_kt = s // 128  # key tiles
    scale = 1.0 / np.sqrt(np.float32(DEPTH))

    mm_cast = lambda ap: ap

    from contextlib import ExitStack

    with tile.TileContext(nc) as tc, ExitStack() as ctx:
        # ---- pools ----
        persist = ctx.enter_context(tc.tile_pool(name="persist", bufs=1))
        xin = ctx.enter_context(tc.tile_pool(name="xin", bufs=6))
        vtp = ctx.enter_context(tc.tile_pool(name="vtp", bufs=2))
        natp = ctx.enter_context(tc.tile_pool(name="natp", bufs=2))
        sbtp = ctx.enter_context(tc.tile_pool(name="sbtp", bufs=3))
        smallp = ctx.enter_context(tc.tile_pool(name="smallp", bufs=2))
        pp_acc = ctx.enter_context(tc.tile_pool(name="pp_acc", bufs=2, space="PSUM"))
        pp_acc2 = ctx.enter_context(tc.tile_pool(name="pp_acc2", bufs=2, space="PSUM"))
        pp_av = ctx.enter_context(tc.tile_pool(name="pp_av", bufs=2, space="PSUM"))

        # ---- constants / weights ----
        ident = persist.tile([128, 128], FP32, tag="ident", name="ident")
        make_identity(nc, ident[:])

        w_sb = {}
        for name, wap in (("q", wq), ("k", wk), ("v", wv)):
            t = persist.tile([128, n_dt, DSL], MDT, tag=f"w{name}", name=f"w{name}_sb")
            nc.sync.dma_start(t[:], wap.rearrange("(t p) n -> p t n", p=128))
            w_sb[name] = t
        b_sb = {}
        for name, bap in (("q", bq), ("k", bk), ("v", bv)):
            t = persist.tile([DSL, 1], FP32, tag=f"b{name}", name=f"b{name}_sb")
            nc.sync.dma_start(t[:], bap)
            b_sb[name] = t
        wo_sb = []
        for h in range(HPC):
            t = persist.tile([DEPTH, d], MDT, tag=f"wo{h}", name=f"wo{h}_sb")
            nc.sync.dma_start(t[:], wo[h * DEPTH : (h + 1) * DEPTH, :])
            wo_sb.append(t)

        # persistent per-batch projected tensors
        QTs = [persist.tile([DSL, s], MDT, tag=f"qt{bi}", name=f"qt{bi}_sb") for bi in range(b)]
        KTs = [persist.tile([DSL, s], MDT, tag=f"kt{bi}", name=f"kt{bi}_sb") for bi in range(b)]
        # V natural [ktok, d] per (b, h), 16 tiles of [128, 65]; col 64 = ones
        Vn = [
            [persist.tile([128, n_kt, DEPTH + 1], MDT, tag=f"vn{bi}{h}", name=f"vn{bi}{h}_sb")
             for h in range(HPC)]
            for bi in range(b)
        ]
        # normalized attnV^T [d, s] per (b, h)
        avn = [
            [persist.tile([DEPTH, s], MDT, tag=f"avn{bi}{h}", name=f"avn{bi}{h}_sb") for h in range(HPC)]
            for bi in range(b)
        ]

        for bi in range(b):
            for h in range(HPC):
                nc.sync.dma_start(Vn[bi][h][:, :, DEPTH : DEPTH + 1], ones)

        # ---- helpers ----
        vt_cur = {}

        def proj_tensor(bi, name, half=None):
            """Project one of q/k/v for batch bi. half=0/1 restricts to one
            half of the sequence (strips are loaded per-half to keep DMA
            bursts small enough to interleave under attention)."""
            xap = {"q": qT, "k": kT, "v": vT}[name]
            if name == "v" and half in (None, 0):
                vt_cur[bi] = vtp.tile([DSL, s], FP32, tag="vt", name="vt_sb")
            vt_sb = vt_cur.get(bi)
            halves = [(0, s // 2), (s // 2, s // 2)] if half is None else (
                [(0, s // 2)] if half == 0 else [(s // 2, s - s // 2)]
            )
            for ho, hl in halves:
                strips = []
                for t in range(n_dt):
                    st = xin.tile([128, s // 2], MDT, tag="strip", name="strip")
                    nc.sync.dma_start(
                        st[:, :hl], xap[bi, t * 128 : (t + 1) * 128, ho : ho + hl]
                    )
                    strips.append(st)
                for co, cl in _chunks(hl, 512):
                    ps = pp_acc.tile([128, 512], FP32, tag="acc", name="acc_ps")
                    for t in range(n_dt):
                        nc.tensor.matmul(
                            ps[:, :cl],
                            mm_cast(w_sb[name][:, t, :]),
                            mm_cast(strips[t][:, co : co + cl]),
                            start=(t == 0),
                            stop=(t == n_dt - 1),
                        )
                    dst = {"q": QTs[bi], "k": KTs[bi], "v": vt_sb}[name]
                    nc.vector.tensor_scalar_add(
                        dst[:, ho + co : ho + co + cl], ps[:, :cl],
                        b_sb[name][:, 0:1],
                    )
                # V^T -> V natural (per head), via PE transpose
                if name == "v":
                    for kt in range(ho // 128, (ho + hl) // 128):
                        pvt = pp_acc.tile([128, 128], FP32, tag="acc", name="accT_ps")
                        nc.tensor.transpose(
                            pvt[:], vt_sb[:, kt * 128 : (kt + 1) * 128], ident[:]
                        )
                        for h in range(HPC):
                            nc.vector.tensor_copy(
                                Vn[bi][h][:, kt, 0:DEPTH],
                                pvt[:, h * DEPTH : (h + 1) * DEPTH],
                            )

        def attn_head(bi, h, chunk_hooks=None):
            chunk_hooks = list(chunk_hooks or [])
            if True:
                qh = QTs[bi][h * DEPTH : (h + 1) * DEPTH, :]
                kh = KTs[bi][h * DEPTH : (h + 1) * DEPTH, :]

                # transposed side: logits^T -> exp -> attnV (+rowsum)
                def t_side(qo, ql):
                    av_ps = pp_av.tile([DEPTH + 1, 512], FP32, tag="av", name="av_ps")
                    for kt2 in range(n_kt // 2):
                        t_ps = pp_acc2.tile([128, 2, 512], FP32, tag="acc2", name="acc2_ps")
                        for j in range(2):
                            kt = 2 * kt2 + j
                            nc.tensor.matmul(
                                t_ps[:, j, :ql],
                                mm_cast(kh[:, kt * 128 : (kt + 1) * 128]),
                                mm_cast(qh[:, qo : qo + ql]),
                                start=True,
                                stop=True,
                            )
                        sbt = sbtp.tile([128, 2, 512], MDT, tag="sbt", name="sbt", bufs=2)
                        nc.scalar.activation(
                            sbt[:, :, :ql], t_ps[:, :, :ql], AF.Exp,
                            scale=float(scale),
                        )
                        for j in range(2):
                            kt = 2 * kt2 + j
                            nc.tensor.matmul(
                                av_ps[:, :ql],
                                mm_cast(Vn[bi][h][:, kt, :]),
                                mm_cast(sbt[:, j, :ql]),
                                start=(kt == 0),
                                stop=(kt == n_kt - 1),
                            )
                    rT = smallp.tile([DEPTH + 1, 512], FP32, tag="rT", name="rT")
                    nc.vector.reciprocal(
                        rT[DEPTH : DEPTH + 1, :ql], av_ps[DEPTH : DEPTH + 1, :ql]
                    )
                    # partition_broadcast ucode only reads partition 0 -> hop
                    # the recip row from partition 64 to 0 via sbuf-sbuf DMA
                    r0 = smallp.tile([1, 512], FP32, tag="r0", name="r0")
                    nc.gpsimd.dma_start(r0[0:1, :ql], rT[DEPTH : DEPTH + 1, :ql])
                    rb = smallp.tile([DEPTH, 512], FP32, tag="rb", name="rb")
                    nc.gpsimd.partition_broadcast(rb[:, :ql], r0[0:1, :ql])
                    nc.vector.tensor_mul(
                        avn[bi][h][:, qo : qo + ql],
                        av_ps[0:DEPTH, :ql],
                        rb[:, :ql],
                    )

                # natural side: logits -> exp(+rowsum) -> normalize -> DMA
                n_ch = _chunks(s, 512)

                def nat_qtile(qt):
                        nexp = natp.tile([128, s], FP32, tag="nexp", name="nexp")
                        nrm = natp.tile([128, s], ADT, tag="nrm", name="nrm")
                        n_ch2 = _chunks(s, 1024)
                        rs4 = smallp.tile([128, len(n_ch2)], FP32, tag="rs4", name="rs4")
                        for ci, (co, cl) in enumerate(n_ch2):
                            n_ps = pp_acc2.tile([128, 2, 512], FP32, tag="acc2", name="acc2_ps")
                            for j in range(cl // 512):
                                nc.tensor.matmul(
                                    n_ps[:, j, :],
                                    mm_cast(qh[:, qt * 128 : (qt + 1) * 128]),
                                    mm_cast(kh[:, co + j * 512 : co + (j + 1) * 512]),
                                    start=True,
                                    stop=True,
                                )
                            nc.scalar.activation(
                                nexp[:, co : co + cl].rearrange("p (a c) -> p a c", c=512),
                                n_ps[:, : cl // 512, :], AF.Exp,
                                scale=float(scale),
                                accum_out=rs4[:, ci : ci + 1],
                            )
                        rs1 = smallp.tile([128, 1], FP32, tag="rs1", name="rs1")
                        if len(n_ch2) > 1:
                            nc.vector.reduce_sum(
                                rs1[:], rs4[:], axis=mybir.AxisListType.X
                            )
                        else:
                            rs1 = rs4
                        rr = smallp.tile([128, 1], FP32, tag="rr", name="rr")
                        nc.vector.reciprocal(rr[:], rs1[:, 0:1])
                        nc.vector.tensor_scalar_mul(nrm[:], nexp[:], rr[:, 0:1])
                        nc.gpsimd.dma_start(
                            attn_w[bi, h, qt * 128 : (qt + 1) * 128, :], nrm[:]
                        )

                qts_per_chunk = (s // 128) // len(_chunks(s, 512))
                for ci, (qo, ql) in enumerate(_chunks(s, 512)):
                    t_side(qo, ql)
                    if nat_side:
                        for qt in range(ci * qts_per_chunk, (ci + 1) * qts_per_chunk):
                            nat_qtile(qt)
                    hook = chunk_hooks.pop(0) if chunk_hooks else None
                    if hook is not None:
                        hook()

        def outproj_batch(bi):
            for tt in range(s // 128):
                for co, cl in _chunks(d, 512):
                    po = pp_acc.tile([128, 512], FP32, tag="acc", name="po_ps")
                    for h in range(HPC):
                        nc.tensor.matmul(
                            po[:, :cl],
                            mm_cast(avn[bi][h][:, tt * 128 : (tt + 1) * 128]),
                            mm_cast(wo_sb[h][:, co : co + cl]),
                            start=(h == 0),
                            stop=(h == HPC - 1),
                        )
                    po_sb = sbtp.tile([128, 512], FP32, tag="po_sb", name="po_sb")
                    nc.vector.tensor_copy(po_sb[:, :cl], po[:, :cl])
                    nc.sync.dma_start(
                        out_p[bi, tt * 128 : (tt + 1) * 128, co : co + cl],
                        po_sb[:, :cl],
                    )

        for nm in ("k", "q", "v"):
            proj_tensor(0, nm)
        if b > 1:
            attn_head(0, 0, [
                lambda: proj_tensor(1, "k", 0),
                lambda: proj_tensor(1, "k", 1),
                lambda: proj_tensor(1, "q", 0),
                lambda: proj_tensor(1, "q", 1),
            ])
            attn_head(0, 1, [
                lambda: proj_tensor(1, "v", 0),
                lambda: proj_tensor(1, "v", 1),
            ])
            outproj_batch(0)
            attn_head(1, 0)
            attn_head(1, 1)
            outproj_batch(1)
        else:
            attn_head(0, 0)
            attn_head(0, 1)
            outproj_batch(0)

    nc.compile()
    return nc


_NC_CACHE = {}
LAST_EXEC_NS = None


def _get_program():
    attn_bf16 = not int(os.environ.get("KERNEL_ATTN_FP32", "0"))
    key = (B, S, D, attn_bf16)
    if key not in _NC_CACHE:
        _NC_CACHE[key] = build_program(attn_bf16=attn_bf16)
    return _NC_CACHE[key]


def kernel(**inputs):
    q = np.asarray(inputs["q"], np.float32)
    k = np.asarray(inputs["k"], np.float32)
    v = np.asarray(inputs["v"], np.float32)
    w_q = np.asarray(inputs["w_q"], np.float32)
    w_k = np.asarray(inputs["w_k"], np.float32)
    w_v = np.asarray(inputs["w_v"], np.float32)
    b_q = np.asarray(inputs["b_q"], np.float32)
    b_k = np.asarray(inputs["b_k"], np.float32)
    b_v = np.asarray(inputs["b_v"], np.float32)
    w_o = np.asarray(inputs["w_o"], np.float32)
    b_o = np.asarray(inputs["b_o"], np.float32)

    qT = np.ascontiguousarray(q.transpose(0, 2, 1))
    kTn = np.ascontiguousarray(k.transpose(0, 2, 1))
    vTn = np.ascontiguousarray(v.transpose(0, 2, 1))

    in_maps = []
    for c in range(N_CORES):
        sl = slice(c * DSL, (c + 1) * DSL)
        in_maps.append(
            {
                "qT": qT,
                "kT": kTn,
                "vT": vTn,
                "wq": np.ascontiguousarray(w_q[:, sl]),
                "wk": np.ascontiguousarray(w_k[:, sl]),
                "wv": np.ascontiguousarray(w_v[:, sl]),
                "bq": np.ascontiguousarray(b_q[sl, None]),
                "bk": np.ascontiguousarray(b_k[sl, None]),
                "bv": np.ascontiguousarray(b_v[sl, None]),
                "wo": np.ascontiguousarray(w_o[sl, :]),
                "ones": np.ones((128, S // 128, 1), np.float32),
            }
        )

    nc = _get_program()
    if int(os.environ.get("KERNEL_SLOW_RUNNER", "0")):
        from concourse.bass_utils import run_bass_kernel_spmd

        r = run_bass_kernel_spmd(nc, in_maps, core_ids=list(range(N_CORES)))
        res = r.results
        attn = np.empty((B, N_HEADS, S, S), np.float32)
        acc = np.zeros((B, S, D), np.float64)
        for c in range(N_CORES):
            attn[:, c * HPC : (c + 1) * HPC] = res[c]["attn_w"]
            acc += res[c]["out_p"]
        output = (acc + b_o.astype(np.float64)).astype(np.float32)
        return output, attn

    try:
        attn_g, outp = _run_fast(nc, in_maps)
    except Exception:
        # fall back to the stock SPMD runner on any fast-path failure
        from concourse.bass_utils import run_bass_kernel_spmd

        res = run_bass_kernel_spmd(nc, in_maps, core_ids=list(range(N_CORES))).results
        attn = np.empty((B, N_HEADS, S, S), np.float32)
        acc = np.zeros((B, S, D), np.float64)
        for c in range(N_CORES):
            attn[:, c * HPC : (c + 1) * HPC] = np.asarray(
                res[c]["attn_w"]
            ).astype(np.float32)
            acc += res[c]["out_p"]
        output = (acc + b_o.astype(np.float64)).astype(np.float32)
        return output, attn
    attn = np.ascontiguousarray(
        attn_g.reshape(N_CORES, B, HPC, S, S).transpose(1, 0, 2, 3, 4).reshape(
            B, N_HEADS, S, S
        ).astype(np.float32, copy=False)
    )
    output = (outp.astype(np.float64) + b_o.astype(np.float64)).astype(np.float32)
    return output, attn


_FAST_CACHE = {}


def _run_fast(nc, in_maps):
    """Axon PJRT runner tuned for transfer volume: replicated inputs are
    shipped once and broadcast on-device, donated output buffers are
    created on-device, and the per-core output partials are reduced
    on-device so only one copy comes back."""
    import jax
    import jax.numpy as jnp
    from jax.sharding import Mesh, NamedSharding, PartitionSpec as P
    from jax.experimental.shard_map import shard_map
    from concourse import bass2jax
    from concourse.bass2jax import install_neuronx_cc_hook
    import concourse.mybir as mybir

    install_neuronx_cc_hook()

    REPL = ("qT", "kT", "vT", "ones")

    in_names = []
    out_names = []
    out_avals = []
    part_name = nc.partition_id_tensor.name if nc.partition_id_tensor else None
    for alloc in nc.m.functions[0].allocations:
        if not isinstance(alloc, mybir.MemoryLocationSet):
            continue
        name = alloc.memorylocations[0].name
        if alloc.kind == "ExternalInput":
            if name != part_name:
                in_names.append(name)
        elif alloc.kind == "ExternalOutput":
            out_names.append(name)
            out_avals.append(
                jax.core.ShapedArray(tuple(alloc.tensor_shape), mybir.dt.np(alloc.dtype))
            )
    n_params = len(in_names)
    all_names = in_names + out_names
    if part_name is not None:
        all_names = all_names + [part_name]

    devices = jax.devices()[:N_CORES]
    mesh = Mesh(np.asarray(devices), ("core",))
    shard = NamedSharding(mesh, P("core"))
    repl = NamedSharding(mesh, P())

    def _body(*args):
        operands = list(args)
        if part_name is not None:
            operands.append(bass2jax.partition_id_tensor())
        outs = bass2jax._bass_exec_p.bind(
            *operands,
            out_avals=tuple(out_avals),
            in_names=tuple(all_names),
            out_names=tuple(out_names),
            lowering_input_output_aliases=(),
            sim_require_finite=True,
            sim_require_nnan=True,
            nc=nc,
        )
        return tuple(outs)

    in_specs = tuple(P() if n in REPL else P("core") for n in in_names) + tuple(
        P("core") for _ in out_names
    )
    out_specs = tuple(P("core") for _ in out_names)
    donate = tuple(range(n_params, n_params + len(out_names)))
    sharded = _FAST_CACHE.get("jit")
    if sharded is None:
        sharded = jax.jit(
            shard_map(
                _body, mesh=mesh, in_specs=in_specs, out_specs=out_specs,
                check_rep=False,
            ),
            donate_argnums=donate,
            keep_unused=True,
        )
        _FAST_CACHE["jit"] = sharded

    # stage inputs; replicated big tensors go over the wire once (sharded on a
    # leading axis of 8) and are all-gathered on device into replicated form
    gat = _FAST_CACHE.get("gather_jit")
    if gat is None:
        def _gather(x):
            # x: [8, b, d/8, s] sharded on axis 0 -> [b, d, s] replicated
            xb, bb, dd, ss = x.shape
            return jnp.transpose(x, (1, 0, 2, 3)).reshape(bb, xb * dd, ss)
        gat = jax.jit(_gather, out_shardings=repl)
        _FAST_CACHE["gather_jit"] = gat

    args = []
    for n in in_names:
        if n in REPL:
            v = in_maps[0][n]
            if v.ndim == 3 and v.shape[1] % N_CORES == 0 and v.nbytes >= 2**22:
                piece = v.shape[1] // N_CORES
                vp = np.ascontiguousarray(
                    v.reshape(v.shape[0], N_CORES, piece, v.shape[2]).transpose(
                        1, 0, 2, 3
                    )
                )
                args.append(gat(jax.device_put(vp, shard)))
            else:
                args.append(jax.device_put(v, repl))
        else:
            cat = np.concatenate([in_maps[c][n] for c in range(N_CORES)], axis=0)
            args.append(jax.device_put(cat, shard))
    # donated zero output buffers, created on device
    zmaker = _FAST_CACHE.get("zeros_jit")
    if zmaker is None:
        def _mk():
            return tuple(
                jnp.zeros((N_CORES * av.shape[0], *av.shape[1:]), av.dtype)
                for av in out_avals
            )
        zmaker = jax.jit(_mk, out_shardings=tuple(shard for _ in out_avals))
        _FAST_CACHE["zeros_jit"] = zmaker
    zeros = zmaker()

    outs = sharded(*args, *zeros)
    out_by_name = dict(zip(out_names, outs))

    # on-device reduction of the per-core output partials
    red = _FAST_CACHE.get("red_jit")
    if red is None:
        def _red(x):
            return jnp.sum(x.reshape(N_CORES, B, S, D), axis=0)
        red = jax.jit(_red, out_shardings=NamedSharding(mesh, P()))
        _FAST_CACHE["red_jit"] = red
    outp_dev = red(out_by_name["out_p"])

    attn_g = np.asarray(out_by_name["attn_w"])
    outp = np.asarray(outp_dev.addressable_shards[0].data)
    return attn_g, outp
